# revision 1
# baseline (speedup 1.0000x reference)
"""minGRU cell kernel for 8 Trainium2 NeuronCores.

Math (per batch b, all in linear domain — the recurrence is a convex
combination of positive values, so no log-space is needed):
    gh[s, :] = x[s, :] @ W.T + b          # (S, 2H)
    gate, hidden = gh[:, :H], gh[:, H:]
    z = sigmoid(gate);  a = 1 - z = sigmoid(-gate)
    g(hidden) = relu(hidden) + min(sigmoid(hidden), 0.5)
    h_t = a_t * h_{t-1} + z_t * g_t       # scan over s

Distribution: pure data parallel over B (8 batches -> 8 cores).

Device layout: channels on SBUF partitions, time on the free dim:
    out[o, s] = sum_i WT[i, o] * xT[i, s]
so the matmul result lands directly in the layout the DVE
tensor_tensor_scan instruction needs.  x / W / output transposes are
done host-side (free w.r.t. HW kernel time).  Matmuls run as float32r
(full-rate fp32 PE mode).
"""

from contextlib import ExitStack

import numpy as np

import concourse.bass as bass
import concourse.bacc as bacc
import concourse.mybir as mybir
import concourse.tile as tile
from concourse.bass_utils import run_bass_kernel_spmd

B, S, DIN, DH = 8, 4096, 1024, 1024
CH = 512                 # time-chunk (free dim of each matmul / scan)
NCHUNK = S // CH         # 8
KT = DIN // 128          # 8 contraction tiles
JT = DH // 128           # 8 channel tiles (per gate/hidden half)

F32 = mybir.dt.float32
F32R = mybir.dt.float32r
AF = mybir.ActivationFunctionType
OP = mybir.AluOpType

_prog_cache = {}


def _build_program() -> bass.Bass:
    nc = bacc.Bacc("TRN2", target_bir_lowering=False, debug=False,
                   num_devices=B)
    xt = nc.dram_tensor("xt", (DIN, S), F32R, kind="ExternalInput")
    wt = nc.dram_tensor("wt", (DIN, 2 * DH), F32R, kind="ExternalInput")
    bias = nc.dram_tensor("bias", (128, 2 * JT), F32, kind="ExternalInput")
    nbias = nc.dram_tensor("nbias", (128, 2 * JT), F32, kind="ExternalInput")
    h0 = nc.dram_tensor("h0", (128, JT), F32, kind="ExternalInput")
    out = nc.dram_tensor("out", (DH, S), F32, kind="ExternalOutput")

    with ExitStack() as ctx:
        tc = ctx.enter_context(tile.TileContext(nc))
        cpool = ctx.enter_context(tc.tile_pool(name="const", bufs=1))
        wpool = ctx.enter_context(tc.tile_pool(name="w", bufs=1))
        xpool = ctx.enter_context(tc.tile_pool(name="x", bufs=2))
        spool = ctx.enter_context(tc.tile_pool(name="tmp", bufs=2))
        abpool = ctx.enter_context(tc.tile_pool(name="ab", bufs=3))
        hpool = ctx.enter_context(tc.tile_pool(name="h", bufs=2))
        ppool = ctx.enter_context(tc.tile_pool(name="psum", bufs=4, space="PSUM"))

        # DMA consts, then re-materialize on the engines that consume them
        # (ACT for bias/nbias, DVE for h0) so hot-loop instructions never
        # carry a DMA sync-wait (codegen has a tight per-instruction limit).
        bias_d = cpool.tile([128, 2 * JT], F32, tag="bias_d")
        nc.sync.dma_start(bias_d[:], bias[:, :])
        nbias_d = cpool.tile([128, 2 * JT], F32, tag="nbias_d")
        nc.sync.dma_start(nbias_d[:], nbias[:, :])
        h0_d = cpool.tile([128, JT], F32, tag="h0_d")
        nc.sync.dma_start(h0_d[:], h0[:, :])
        bias_t = cpool.tile([128, 2 * JT], F32, tag="bias")
        nc.scalar.copy(bias_t[:], bias_d[:])
        nbias_t = cpool.tile([128, 2 * JT], F32, tag="nbias")
        nc.scalar.copy(nbias_t[:], nbias_d[:])
        h0_t = cpool.tile([128, JT], F32, tag="h0")
        nc.vector.tensor_copy(h0_t[:], h0_d[:])

        wts = []
        for k in range(KT):
            w_t = wpool.tile([128, 2 * DH], F32R, tag=f"w{k}")
            nc.sync.dma_start(w_t[:], wt[k * 128:(k + 1) * 128, :])
            wts.append(w_t)

        prev_h = [None] * JT
        for c in range(NCHUNK):
            s0 = c * CH
            xts = []
            for k in range(KT):
                x_t = xpool.tile([128, CH], F32R, tag=f"x{k}")
                nc.sync.dma_start(x_t[:], xt[k * 128:(k + 1) * 128, s0:s0 + CH])
                xts.append(x_t)
            for j in range(JT):
                # ---- gate channel-tile j: psum = (W.T x)[j*128:(j+1)*128, s0:s0+CH]
                pg = ppool.tile([128, CH], F32, tag="psum")
                for k in range(KT):
                    nc.tensor.matmul(
                        pg[:],
                        lhsT=wts[k][:, j * 128:(j + 1) * 128],
                        rhs=xts[k][:],
                        start=(k == 0),
                        stop=(k == KT - 1),
                    )
                a_t = abpool.tile([128, CH], F32, tag="a")
                nc.scalar.activation(a_t[:], pg[:], AF.Sigmoid,
                                     bias=nbias_t[:, j:j + 1], scale=-1.0)
                z_t = spool.tile([128, CH], F32, tag="z")
                nc.scalar.activation(z_t[:], pg[:], AF.Sigmoid,
                                     bias=bias_t[:, j:j + 1], scale=1.0)
                # ---- hidden channel-tile j (o-tile JT+j)
                ph = ppool.tile([128, CH], F32, tag="psum")
                for k in range(KT):
                    nc.tensor.matmul(
                        ph[:],
                        lhsT=wts[k][:, (JT + j) * 128:(JT + j + 1) * 128],
                        rhs=xts[k][:],
                        start=(k == 0),
                        stop=(k == KT - 1),
                    )
                sg_t = spool.tile([128, CH], F32, tag="sg")
                nc.scalar.activation(sg_t[:], ph[:], AF.Sigmoid,
                                     bias=bias_t[:, JT + j:JT + j + 1], scale=1.0)
                r_t = spool.tile([128, CH], F32, tag="r")
                nc.scalar.activation(r_t[:], ph[:], AF.Relu,
                                     bias=bias_t[:, JT + j:JT + j + 1], scale=1.0)
                # g = min(sigmoid(hidden), 0.5) + relu(hidden)
                g_t = spool.tile([128, CH], F32, tag="g")
                nc.vector.scalar_tensor_tensor(g_t[:], sg_t[:], 0.5, r_t[:],
                                               op0=OP.min, op1=OP.add)
                b_t = abpool.tile([128, CH], F32, tag="b")
                nc.vector.tensor_mul(b_t[:], z_t[:], g_t[:])
                # ---- scan: h = a*h_prev + b along time
                h_t = hpool.tile([128, CH], F32, tag=f"h{j}")
                init = h0_t[:, j:j + 1] if c == 0 else prev_h[j][:, CH - 1:CH]
                nc.vector.tensor_tensor_scan(h_t[:], a_t[:], b_t[:], init,
                                             op0=OP.mult, op1=OP.add)
                prev_h[j] = h_t
                nc.sync.dma_start(out[j * 128:(j + 1) * 128, s0:s0 + CH], h_t[:])
    nc.compile()
    return nc


def _run(inputs, trace=False, **spmd_kwargs):
    x = np.asarray(inputs["x"], dtype=np.float32)
    h = np.asarray(inputs["h"], dtype=np.float32)
    W = np.asarray(inputs["W"], dtype=np.float32)
    b = np.asarray(inputs["b"], dtype=np.float32)

    xt_all = np.ascontiguousarray(x.transpose(0, 2, 1))            # (B, DIN, S)
    WT = np.ascontiguousarray(W.T)                                 # (DIN, 2DH)
    bias_t = np.ascontiguousarray(b.reshape(2 * JT, 128).T)        # (128, 2JT)
    nbias_t = np.ascontiguousarray(-bias_t)
    h0_all = np.ascontiguousarray(
        h[:, 0, :].reshape(B, JT, 128).transpose(0, 2, 1))         # (B, 128, JT)

    if "prog" not in _prog_cache:
        _prog_cache["prog"] = _build_program()
    nc = _prog_cache["prog"]

    in_maps = [
        {"xt": xt_all[c], "wt": WT, "bias": bias_t, "nbias": nbias_t,
         "h0": h0_all[c]}
        for c in range(B)
    ]
    res = run_bass_kernel_spmd(nc, in_maps, list(range(B)), trace=trace,
                               **spmd_kwargs)
    out = np.stack([res.results[c]["out"].T for c in range(B)], axis=0)
    return np.ascontiguousarray(out), res


def kernel(**inputs) -> np.ndarray:
    return _run(inputs)[0]



# revision 3
# speedup vs baseline: 1.0057x; 1.0057x over previous
"""minGRU cell kernel for 8 Trainium2 NeuronCores.

Math (per batch b, all in linear domain — the recurrence is a convex
combination of positive values, so no log-space is needed):
    gh[s, :] = x[s, :] @ W.T + b          # (S, 2H)
    gate, hidden = gh[:, :H], gh[:, H:]
    z = sigmoid(gate);  a = 1 - z = sigmoid(-gate)
    g(hidden) = relu(hidden) + min(sigmoid(hidden), 0.5)
    h_t = a_t * h_{t-1} + z_t * g_t       # scan over s

Distribution: pure data parallel over B (8 batches -> 8 cores).

Device layout: channels on SBUF partitions, time on the free dim:
    out[o, s] = sum_i WT[i, o] * xT[i, s]
so the matmul result lands directly in the layout the DVE
tensor_tensor_scan instruction needs.  Matmuls run as float32r
(full-rate fp32 PE mode).

Perf structure (vs the naive version):
  * W is packed host-side per output-tile (o-major), so the first
    j-chain only needs 1 MB of W + 2.1 MB of x before the PE can
    start — the old k-major layout needed all 8.4 MB of W first
    (26 us of PE idle at startup).
  * DMA descriptor issue is spread across three queues (W on the
    Vector queue, x on Sync, consts + output stores on GpSimd) so
    descriptor serialization never gates the startup transfers.
  * Within each (chunk, j) iteration the hidden chain runs BEFORE the
    gate chain: the post-matmul serial tail (sigmoid/relu/min-add on
    the hidden projection) overlaps the gate matmuls, and the final
    chunk's post-ops are split into 256-column halves, shortening the
    end-of-kernel drain.
"""

from contextlib import ExitStack

import numpy as np

import concourse.bass as bass
import concourse.bacc as bacc
import concourse.mybir as mybir
import concourse.tile as tile
from concourse.bass_utils import run_bass_kernel_spmd

B, S, DIN, DH = 8, 4096, 1024, 1024
CH = 512                 # time-chunk (free dim of each matmul / scan)
NCHUNK = S // CH         # 8
KT = DIN // 128          # 8 contraction tiles
JT = DH // 128           # 8 channel tiles (per gate/hidden half)

F32 = mybir.dt.float32
F32R = mybir.dt.float32r
AF = mybir.ActivationFunctionType
OP = mybir.AluOpType

_prog_cache = {}


def _build_program() -> bass.Bass:
    nc = bacc.Bacc("TRN2", target_bir_lowering=False, debug=False,
                   num_devices=B)
    xt = nc.dram_tensor("xt", (DIN, S), F32R, kind="ExternalInput")
    # per-o packed weights: wp[o*128+p, k*128+c] = W[o*128+c, k*128+p]
    wp = nc.dram_tensor("wp", (2 * DH, DIN), F32R, kind="ExternalInput")
    bias = nc.dram_tensor("bias", (128, 2 * JT), F32, kind="ExternalInput")
    nbias = nc.dram_tensor("nbias", (128, 2 * JT), F32, kind="ExternalInput")
    h0 = nc.dram_tensor("h0", (128, JT), F32, kind="ExternalInput")
    out = nc.dram_tensor("out", (DH, S), F32, kind="ExternalOutput")

    with ExitStack() as ctx:
        tc = ctx.enter_context(tile.TileContext(nc))
        cpool = ctx.enter_context(tc.tile_pool(name="const", bufs=1))
        wpool = ctx.enter_context(tc.tile_pool(name="w", bufs=1))
        xpool = ctx.enter_context(tc.tile_pool(name="x", bufs=2))
        spool = ctx.enter_context(tc.tile_pool(name="tmp", bufs=2))
        abpool = ctx.enter_context(tc.tile_pool(name="ab", bufs=3))
        hpool = ctx.enter_context(tc.tile_pool(name="h", bufs=2))
        ppool = ctx.enter_context(tc.tile_pool(name="psum", bufs=4, space="PSUM"))

        # Consts go over the (otherwise idle) GpSimd queue, then are
        # re-materialized on the engines that consume them (ACT for
        # bias/nbias, DVE for h0) so hot-loop instructions never carry
        # a DMA sync-wait.
        bias_d = cpool.tile([128, 2 * JT], F32, tag="bias_d")
        nc.gpsimd.dma_start(bias_d[:], bias[:, :])
        nbias_d = cpool.tile([128, 2 * JT], F32, tag="nbias_d")
        nc.gpsimd.dma_start(nbias_d[:], nbias[:, :])
        h0_d = cpool.tile([128, JT], F32, tag="h0_d")
        nc.gpsimd.dma_start(h0_d[:], h0[:, :])
        bias_t = cpool.tile([128, 2 * JT], F32, tag="bias")
        nc.scalar.copy(bias_t[:], bias_d[:])
        nbias_t = cpool.tile([128, 2 * JT], F32, tag="nbias")
        nc.scalar.copy(nbias_t[:], nbias_d[:])
        h0_t = cpool.tile([128, JT], F32, tag="h0")
        nc.vector.tensor_copy(h0_t[:], h0_d[:])

        # W on the Scalar queue (idle at startup), ordered so the first
        # j-chains' tiles (hidden o=8+j first: the hidden chain runs
        # first) land first.
        wts = [None] * (2 * JT)
        for j in range(JT):
            for o in (JT + j, j):
                w_t = wpool.tile([128, DIN], F32R, tag=f"w{o}")
                nc.scalar.dma_start(w_t[:], wp[o * 128:(o + 1) * 128, :])
                wts[o] = w_t

        prev_h = [None] * JT
        for c in range(NCHUNK):
            s0 = c * CH
            sub = CH if c < NCHUNK - 1 else CH // 2  # split last chunk's post-ops
            xts = []
            for k in range(KT):
                x_t = xpool.tile([128, CH], F32R, tag=f"x{k}")
                nc.sync.dma_start(x_t[:], xt[k * 128:(k + 1) * 128, s0:s0 + CH])
                xts.append(x_t)
            for j in range(JT):
                # ---- hidden channel-tile j (o-tile JT+j) FIRST
                ph = ppool.tile([128, CH], F32, tag="psum")
                for k in range(KT):
                    nc.tensor.matmul(
                        ph[:],
                        lhsT=wts[JT + j][:, k * 128:(k + 1) * 128],
                        rhs=xts[k][:],
                        start=(k == 0),
                        stop=(k == KT - 1),
                    )
                gs = []
                for f0 in range(0, CH, sub):
                    fs = slice(f0, f0 + sub)
                    sg_t = spool.tile([128, sub], F32, tag="sg")
                    nc.scalar.activation(sg_t[:], ph[:, fs], AF.Sigmoid,
                                         bias=bias_t[:, JT + j:JT + j + 1],
                                         scale=1.0)
                    r_t = spool.tile([128, sub], F32, tag="r")
                    nc.scalar.activation(r_t[:], ph[:, fs], AF.Relu,
                                         bias=bias_t[:, JT + j:JT + j + 1],
                                         scale=1.0)
                    # g = min(sigmoid(hidden), 0.5) + relu(hidden)
                    g_t = spool.tile([128, sub], F32, tag="g")
                    nc.vector.scalar_tensor_tensor(g_t[:], sg_t[:], 0.5, r_t[:],
                                                   op0=OP.min, op1=OP.add)
                    gs.append(g_t)
                # ---- gate channel-tile j (overlaps the hidden post-ops)
                pg = ppool.tile([128, CH], F32, tag="psum")
                for k in range(KT):
                    nc.tensor.matmul(
                        pg[:],
                        lhsT=wts[j][:, k * 128:(k + 1) * 128],
                        rhs=xts[k][:],
                        start=(k == 0),
                        stop=(k == KT - 1),
                    )
                h_t = hpool.tile([128, CH], F32, tag=f"h{j}")
                for i, f0 in enumerate(range(0, CH, sub)):
                    fs = slice(f0, f0 + sub)
                    a_t = abpool.tile([128, sub], F32, tag="a")
                    nc.scalar.activation(a_t[:], pg[:, fs], AF.Sigmoid,
                                         bias=nbias_t[:, j:j + 1], scale=-1.0)
                    z_t = spool.tile([128, sub], F32, tag="z")
                    nc.scalar.activation(z_t[:], pg[:, fs], AF.Sigmoid,
                                         bias=bias_t[:, j:j + 1], scale=1.0)
                    b_t = abpool.tile([128, sub], F32, tag="b")
                    nc.vector.tensor_mul(b_t[:], z_t[:], gs[i][:])
                    # ---- scan: h = a*h_prev + b along time
                    if i == 0:
                        init = (h0_t[:, j:j + 1] if c == 0
                                else prev_h[j][:, CH - 1:CH])
                    else:
                        init = h_t[:, f0 - 1:f0]
                    nc.vector.tensor_tensor_scan(h_t[:, fs], a_t[:], b_t[:],
                                                 init, op0=OP.mult, op1=OP.add)
                prev_h[j] = h_t
                nc.gpsimd.dma_start(out[j * 128:(j + 1) * 128, s0:s0 + CH],
                                    h_t[:])
    nc.compile()
    return nc


def _run(inputs, trace=False, **spmd_kwargs):
    x = np.asarray(inputs["x"], dtype=np.float32)
    h = np.asarray(inputs["h"], dtype=np.float32)
    W = np.asarray(inputs["W"], dtype=np.float32)
    b = np.asarray(inputs["b"], dtype=np.float32)

    xt_all = np.ascontiguousarray(x.transpose(0, 2, 1))            # (B, DIN, S)
    # wp[o*128+p, k*128+c] = W[o*128+c, k*128+p]
    WP = np.ascontiguousarray(
        W.reshape(2 * JT, 128, KT, 128).transpose(0, 3, 2, 1)
        .reshape(2 * DH, DIN))
    bias_t = np.ascontiguousarray(b.reshape(2 * JT, 128).T)        # (128, 2JT)
    nbias_t = np.ascontiguousarray(-bias_t)
    h0_all = np.ascontiguousarray(
        h[:, 0, :].reshape(B, JT, 128).transpose(0, 2, 1))         # (B, 128, JT)

    if "prog" not in _prog_cache:
        _prog_cache["prog"] = _build_program()
    nc = _prog_cache["prog"]

    in_maps = [
        {"xt": xt_all[c], "wp": WP, "bias": bias_t, "nbias": nbias_t,
         "h0": h0_all[c]}
        for c in range(B)
    ]
    res = run_bass_kernel_spmd(nc, in_maps, list(range(B)), trace=trace,
                               **spmd_kwargs)
    out = np.stack([res.results[c]["out"].T for c in range(B)], axis=0)
    return np.ascontiguousarray(out), res


def kernel(**inputs) -> np.ndarray:
    return _run(inputs)[0]


# revision 6
# speedup vs baseline: 1.0184x; 1.0126x over previous
"""minGRU cell kernel for 8 Trainium2 NeuronCores.

Math (per batch b, all in linear domain — the recurrence is a convex
combination of positive values, so no log-space is needed):
    gh[s, :] = x[s, :] @ W.T + b          # (S, 2H)
    gate, hidden = gh[:, :H], gh[:, H:]
    z = sigmoid(gate);  a = 1 - z = sigmoid(-gate)
    g(hidden) = relu(hidden) + min(sigmoid(hidden), 0.5)
    h_t = a_t * h_{t-1} + z_t * g_t       # scan over s

Distribution: pure data parallel over B (8 batches -> 8 cores).

Device layout: channels on SBUF partitions, time on the free dim:
    out[o, s] = sum_i WT[i, o] * xT[i, s]
so the matmul result lands directly in the layout the DVE
tensor_tensor_scan instruction needs.  Matmuls run as float32r
(full-rate fp32 PE mode).

Perf structure (vs the naive version):
  * W is packed host-side per output-tile (o-major), so the first
    j-chain only needs 1 MB of W + 2.1 MB of x before the PE can
    start — the old k-major layout needed all 8.4 MB of W first
    (26 us of PE idle at startup).
  * DMA descriptor issue is spread across three queues (W on the
    Vector queue, x on Sync, consts + output stores on GpSimd) so
    descriptor serialization never gates the startup transfers.
  * Within each (chunk, j) iteration the hidden chain runs BEFORE the
    gate chain: the post-matmul serial tail (sigmoid/relu/min-add on
    the hidden projection) overlaps the gate matmuls, and the final
    chunk's post-ops are split into 256-column halves, shortening the
    end-of-kernel drain.
"""

from contextlib import ExitStack

import numpy as np

import concourse.bass as bass
import concourse.bacc as bacc
import concourse.mybir as mybir
import concourse.tile as tile
from concourse.bass_utils import run_bass_kernel_spmd

B, S, DIN, DH = 8, 4096, 1024, 1024
CH = 512                 # time-chunk (free dim of each matmul / scan)
NCHUNK = S // CH         # 8
KT = DIN // 128          # 8 contraction tiles
JT = DH // 128           # 8 channel tiles (per gate/hidden half)

F32 = mybir.dt.float32
F32R = mybir.dt.float32r
AF = mybir.ActivationFunctionType
OP = mybir.AluOpType

_prog_cache = {}


def _build_program() -> bass.Bass:
    nc = bacc.Bacc("TRN2", target_bir_lowering=False, debug=False,
                   num_devices=B)
    xt = nc.dram_tensor("xt", (DIN, S), F32R, kind="ExternalInput")
    # per-o packed weights: wp[o*128+p, k*128+c] = W[o*128+c, k*128+p]
    wp = nc.dram_tensor("wp", (2 * DH, DIN), F32R, kind="ExternalInput")
    bias = nc.dram_tensor("bias", (128, 2 * JT), F32, kind="ExternalInput")
    nbias = nc.dram_tensor("nbias", (128, 2 * JT), F32, kind="ExternalInput")
    h0 = nc.dram_tensor("h0", (128, JT), F32, kind="ExternalInput")
    out = nc.dram_tensor("out", (DH, S), F32, kind="ExternalOutput")

    with ExitStack() as ctx:
        tc = ctx.enter_context(tile.TileContext(nc))
        cpool = ctx.enter_context(tc.tile_pool(name="const", bufs=1))
        wpool = ctx.enter_context(tc.tile_pool(name="w", bufs=1))
        xpool = ctx.enter_context(tc.tile_pool(name="x", bufs=2))
        spool = ctx.enter_context(tc.tile_pool(name="tmp", bufs=2))
        abpool = ctx.enter_context(tc.tile_pool(name="ab", bufs=3))
        hpool = ctx.enter_context(tc.tile_pool(name="h", bufs=2))
        ppool = ctx.enter_context(tc.tile_pool(name="psum", bufs=4, space="PSUM"))

        # Consts go over the (otherwise idle) GpSimd queue, then are
        # re-materialized on the engines that consume them (ACT for
        # bias/nbias, DVE for h0) so hot-loop instructions never carry
        # a DMA sync-wait.
        bias_d = cpool.tile([128, 2 * JT], F32, tag="bias_d")
        nc.gpsimd.dma_start(bias_d[:], bias[:, :])
        nbias_d = cpool.tile([128, 2 * JT], F32, tag="nbias_d")
        nc.gpsimd.dma_start(nbias_d[:], nbias[:, :])
        h0_d = cpool.tile([128, JT], F32, tag="h0_d")
        nc.gpsimd.dma_start(h0_d[:], h0[:, :])

        # W on the Scalar queue (idle at startup), issued BEFORE the
        # const copies so the first weight transfer starts immediately.
        # Order = first-use order of the software-pipelined schedule
        # below (hidden o=8+j is used before gate o=j within each j).
        w_order = []
        for j in range(6):
            w_order += [JT + j, j]
        w_order += [JT + 6, 6, JT + 7, 7]
        wts = [None] * (2 * JT)
        for o in w_order:
            w_t = wpool.tile([128, DIN], F32R, tag=f"w{o}")
            nc.scalar.dma_start(w_t[:], wp[o * 128:(o + 1) * 128, :])
            wts[o] = w_t

        bias_t = cpool.tile([128, 2 * JT], F32, tag="bias")
        nc.scalar.copy(bias_t[:], bias_d[:])
        nbias_t = cpool.tile([128, 2 * JT], F32, tag="nbias")
        nc.scalar.copy(nbias_t[:], nbias_d[:])
        h0_t = cpool.tile([128, JT], F32, tag="h0")
        nc.vector.tensor_copy(h0_t[:], h0_d[:])

        # Software-pipelined schedule: W streams in behind the compute
        # (8.4 MB at ~330 GB/s arrives at ~35 us), so the tiles first
        # used by (c0, j6/j7) aren't ready until well into chunk 0.
        # Interleave the first two chunks so those chains run late
        # enough to never stall, while per-j chunk order (the scan
        # dependency) is preserved.
        sched = ([(0, j) for j in range(6)] + [(1, j) for j in range(4)]
                 + [(0, 6), (0, 7)] + [(1, j) for j in range(4, 8)])
        for c in range(2, NCHUNK):
            sched += [(c, j) for j in range(JT)]

        xts_by_chunk = {}
        prev_h = [None] * JT
        for c, j in sched:
            if c not in xts_by_chunk:
                s0 = c * CH
                xts = []
                for k in range(KT):
                    x_t = xpool.tile([128, CH], F32R, tag=f"x{k}")
                    nc.sync.dma_start(x_t[:],
                                      xt[k * 128:(k + 1) * 128, s0:s0 + CH])
                    xts.append(x_t)
                xts_by_chunk[c] = xts
            if True:
                s0 = c * CH
                sub = CH if c < NCHUNK - 1 else CH // 2
                xts = xts_by_chunk[c]
                # ---- hidden channel-tile j (o-tile JT+j) FIRST
                ph = ppool.tile([128, CH], F32, tag="psum")
                for k in range(KT):
                    nc.tensor.matmul(
                        ph[:],
                        lhsT=wts[JT + j][:, k * 128:(k + 1) * 128],
                        rhs=xts[k][:],
                        start=(k == 0),
                        stop=(k == KT - 1),
                    )
                gs = []
                for f0 in range(0, CH, sub):
                    fs = slice(f0, f0 + sub)
                    sg_t = spool.tile([128, sub], F32, tag="sg")
                    nc.scalar.activation(sg_t[:], ph[:, fs], AF.Sigmoid,
                                         bias=bias_t[:, JT + j:JT + j + 1],
                                         scale=1.0)
                    r_t = spool.tile([128, sub], F32, tag="r")
                    nc.scalar.activation(r_t[:], ph[:, fs], AF.Relu,
                                         bias=bias_t[:, JT + j:JT + j + 1],
                                         scale=1.0)
                    # g = min(sigmoid(hidden), 0.5) + relu(hidden)
                    g_t = spool.tile([128, sub], F32, tag="g")
                    nc.vector.scalar_tensor_tensor(g_t[:], sg_t[:], 0.5, r_t[:],
                                                   op0=OP.min, op1=OP.add)
                    gs.append(g_t)
                # ---- gate channel-tile j (overlaps the hidden post-ops)
                pg = ppool.tile([128, CH], F32, tag="psum")
                for k in range(KT):
                    nc.tensor.matmul(
                        pg[:],
                        lhsT=wts[j][:, k * 128:(k + 1) * 128],
                        rhs=xts[k][:],
                        start=(k == 0),
                        stop=(k == KT - 1),
                    )
                h_t = hpool.tile([128, CH], F32, tag=f"h{j}")
                for i, f0 in enumerate(range(0, CH, sub)):
                    fs = slice(f0, f0 + sub)
                    a_t = abpool.tile([128, sub], F32, tag="a")
                    nc.scalar.activation(a_t[:], pg[:, fs], AF.Sigmoid,
                                         bias=nbias_t[:, j:j + 1], scale=-1.0)
                    z_t = spool.tile([128, sub], F32, tag="z")
                    nc.scalar.activation(z_t[:], pg[:, fs], AF.Sigmoid,
                                         bias=bias_t[:, j:j + 1], scale=1.0)
                    b_t = abpool.tile([128, sub], F32, tag="b")
                    nc.vector.tensor_mul(b_t[:], z_t[:], gs[i][:])
                    # ---- scan: h = a*h_prev + b along time
                    if i == 0:
                        init = (h0_t[:, j:j + 1] if c == 0
                                else prev_h[j][:, CH - 1:CH])
                    else:
                        init = h_t[:, f0 - 1:f0]
                    nc.vector.tensor_tensor_scan(h_t[:, fs], a_t[:], b_t[:],
                                                 init, op0=OP.mult, op1=OP.add)
                prev_h[j] = h_t
                # GpSimd's end-of-kernel DRAIN detects DMA completion
                # slowly (~6 us); keep the final chunks' stores on Sync
                # (idle by then) so the kernel end isn't gated on it.
                out_q = nc.gpsimd if c < NCHUNK - 2 else nc.sync
                out_q.dma_start(out[j * 128:(j + 1) * 128, s0:s0 + CH],
                                h_t[:])
    nc.compile()
    return nc


def _run(inputs, trace=False, **spmd_kwargs):
    x = np.asarray(inputs["x"], dtype=np.float32)
    h = np.asarray(inputs["h"], dtype=np.float32)
    W = np.asarray(inputs["W"], dtype=np.float32)
    b = np.asarray(inputs["b"], dtype=np.float32)

    xt_all = np.ascontiguousarray(x.transpose(0, 2, 1))            # (B, DIN, S)
    # wp[o*128+p, k*128+c] = W[o*128+c, k*128+p]
    WP = np.ascontiguousarray(
        W.reshape(2 * JT, 128, KT, 128).transpose(0, 3, 2, 1)
        .reshape(2 * DH, DIN))
    bias_t = np.ascontiguousarray(b.reshape(2 * JT, 128).T)        # (128, 2JT)
    nbias_t = np.ascontiguousarray(-bias_t)
    h0_all = np.ascontiguousarray(
        h[:, 0, :].reshape(B, JT, 128).transpose(0, 2, 1))         # (B, 128, JT)

    if "prog" not in _prog_cache:
        _prog_cache["prog"] = _build_program()
    nc = _prog_cache["prog"]

    in_maps = [
        {"xt": xt_all[c], "wp": WP, "bias": bias_t, "nbias": nbias_t,
         "h0": h0_all[c]}
        for c in range(B)
    ]
    res = run_bass_kernel_spmd(nc, in_maps, list(range(B)), trace=trace,
                               **spmd_kwargs)
    out = np.stack([res.results[c]["out"].T for c in range(B)], axis=0)
    return np.ascontiguousarray(out), res


def kernel(**inputs) -> np.ndarray:
    return _run(inputs)[0]
